# revision 1
# baseline (speedup 1.0000x reference)
"""Trainium2 Bass kernel for nn_CrossAttention_15418932593009.

Reference computation (fp32):
    q = (x @ wq1) @ wq2                      # (b, n, h*d), bottleneck 40
    k = silu(x @ wk1) @ wk2
    v = (x @ wv1) @ wv2
    split '(b n (h d)) -> (b (h n) d)'       # heads folded into sequence!
    sim  = q @ k.T * d**-0.5                 # (b, h*n, h*n) = (4, 8192, 8192)
    attn = softmax(sim, axis=-1)
    out  = attn @ v                          # (b, h*n, d)
    merge back -> (b, n, h*d); out @ wo + bo

Sharding: 8 cores = 4 batches x 2 query-head groups (heads 0-3 / 4-7).
Each core computes full K/V for its batch (all 8 heads) and attention for
its 4 query heads (4096 query rows x 8192 keys), then its partial
contribution of the output projection (its head group's slice of wo).
Host sums the two partials per batch and adds bo.

On-chip layout is "transposed": all SBUF activations keep the contraction
axis on partitions. Scores are computed as S^T tiles (128 keys x 512
queries), exp'd on ACT straight out of PSUM (no max subtraction: scores
are ~N(0, 0.6), softmax is shift-invariant and fp32 exp cannot overflow),
and fed to the A@V matmul which contracts keys on partitions. The softmax
denominator comes free from a ones-column appended to V (M=65 matmul).
Matmuls run in float32r (full PE rate at N>=256, ~tf32 precision); the
BIR verifier requires every fp32r matmul operand to be produced as
float32r, so DRAM inputs are declared float32r and on-chip producers
write float32r tiles.
"""

import numpy as np

HEADS = 8
D = 64
BOT = 40
B = 4
N = 1024
QS = 256
INNER = HEADS * D          # 512
GH = 4                     # query heads per core
KCH = HEADS * N // 128     # 64 key chunks of 128
QT = 512                   # query tile (matmul moving dim)
NQT = GH * N // QT         # 8 query tiles per core
NCORES = 8

_BUILT = {}


def _build():
    """Build the single-core Bass module (same NEFF for all 8 cores)."""
    import concourse.bass as bass
    import concourse.mybir as mybir
    import concourse.tile as tile
    from concourse import bacc

    dt = mybir.dt
    f32 = dt.float32
    f32r = dt.float32r
    AF = mybir.ActivationFunctionType
    PSUM = bass.MemorySpace.PSUM

    # Bacc (not plain Bass): its compile() pipeline moves/splits semaphore
    # waits (TRN2 allows at most 1 wait per instruction; fp32r matmuls
    # self-load weights so waits can't ride a separate LDWEIGHTS).
    nc = bacc.Bacc()

    # ---- DRAM I/O (per core); float32r = same bits as fp32 ----
    xT = nc.dram_tensor("xT", [QS, N], f32r, kind="ExternalInput")      # x[b].T
    wq1 = nc.dram_tensor("wq1", [QS, BOT], f32r, kind="ExternalInput")
    wk1 = nc.dram_tensor("wk1", [QS, BOT], f32r, kind="ExternalInput")
    wv1 = nc.dram_tensor("wv1", [QS, BOT], f32r, kind="ExternalInput")
    wq2g = nc.dram_tensor("wq2g", [BOT, GH * D], f32r, kind="ExternalInput")
    wk2 = nc.dram_tensor("wk2", [BOT, INNER], f32r, kind="ExternalInput")
    wv2 = nc.dram_tensor("wv2", [BOT, INNER], f32r, kind="ExternalInput")
    wog = nc.dram_tensor("wog", [GH * D, QS], f32r, kind="ExternalInput")
    out = nc.dram_tensor("out", [QS, N], f32, kind="ExternalOutput")    # partial^T

    with tile.TileContext(nc) as tc:
        with (
            tc.tile_pool(name="consts", bufs=1) as consts,
            tc.tile_pool(name="bigs", bufs=1) as bigs,
            tc.tile_pool(name="pp", bufs=5) as ppool,
            tc.tile_pool(name="small", bufs=4) as small,
            tc.tile_pool(name="mps", bufs=3, space=PSUM) as mpsum,
            tc.tile_pool(name="avps", bufs=2, space=PSUM) as avpsum,
        ):
            # ---- load inputs (one DMA per tensor: keeps consumer waits few) ----
            xT_sb = bigs.tile([128, 2, N], f32r)
            nc.sync.dma_start(xT_sb[:], xT.rearrange("(c p) n -> p c n", c=2))

            w1_sb = {}
            for name, t in (("q", wq1), ("k", wk1), ("v", wv1)):
                w = consts.tile([128, 2, BOT], f32r, name=f"w{name}1_sb")
                nc.sync.dma_start(w[:], t.rearrange("(c p) n -> p c n", c=2))
                w1_sb[name] = w
            wq2_sb = consts.tile([BOT, GH * D], f32r)
            nc.sync.dma_start(wq2_sb[:], wq2g[:])
            wk2_sb = consts.tile([BOT, INNER], f32r)
            nc.sync.dma_start(wk2_sb[:], wk2[:])
            wv2_sb = consts.tile([BOT, INNER], f32r)
            nc.sync.dma_start(wv2_sb[:], wv2[:])
            wog_sb = consts.tile([128, 2, QS], f32r)
            nc.sync.dma_start(wog_sb[:], wog.rearrange("(c p) n -> p c n", c=2))
            ones_sb = consts.tile([1, D], f32)
            nc.vector.memset(ones_sb[:], 1.0)

            # ---- bottleneck projections: bX^T = wX1^T @ x^T  (40, 1024) ----
            b_sb = {}
            for name in ("q", "k", "v"):
                ps = mpsum.tile([128, 1024], f32, tag="m")
                for s in range(2):
                    for cc in range(2):
                        nc.tensor.matmul(
                            ps[0:BOT, 512 * s : 512 * (s + 1)],
                            w1_sb[name][:, cc, :],
                            xT_sb[:, cc, 512 * s : 512 * (s + 1)],
                            start=(cc == 0),
                            stop=(cc == 1),
                        )
                bt = bigs.tile([BOT, N], f32r, name=f"b{name}_sb")
                if name == "k":
                    # silu(x) = x * sigmoid(x) (Silu table not in CoreSim)
                    sg = bigs.tile([BOT, N], f32, name="sg_sb")
                    nc.scalar.activation(sg[:], ps[0:BOT, 0:N], AF.Sigmoid)
                    nc.vector.tensor_mul(bt[:], ps[0:BOT, 0:N], sg[:])
                elif name == "q":
                    # fold the attention scale d**-0.5 into q
                    nc.vector.tensor_scalar_mul(bt[:], ps[0:BOT, 0:N], D**-0.5)
                else:
                    nc.vector.tensor_copy(bt[:], ps[0:BOT, 0:N])
                b_sb[name] = bt

            # ---- q^T (128, 4096): head-major columns, duplicated on rows
            # 64:128 so row-packed S matmuls can stream rhs at base 64 ----
            qT_sb = bigs.tile([128, GH * N], f32r)
            for hl in range(GH):
                ps = mpsum.tile([128, 1024], f32, tag="m")
                for s in range(2):
                    nc.tensor.matmul(
                        ps[0:D, 512 * s : 512 * (s + 1)],
                        wq2_sb[:, D * hl : D * (hl + 1)],
                        b_sb["q"][:, 512 * s : 512 * (s + 1)],
                    )
                nc.vector.tensor_copy(qT_sb[0:D, N * hl : N * (hl + 1)], ps[0:D, 0:N])
                nc.vector.tensor_copy(
                    qT_sb[D:128, N * hl : N * (hl + 1)], ps[0:D, 0:N]
                )

            # ---- k^T interleaved (128, 4096): column block t holds key
            # chunk 2t on rows 0:64 and chunk 2t+1 on rows 64:128, so pairs
            # of K=64 S matmuls run concurrently on PE row groups ----
            kT_sb = bigs.tile([128, KCH * 64], f32r)
            for hk in range(HEADS):
                ps = mpsum.tile([128, 1024], f32, tag="m")
                for s in range(2):
                    nc.tensor.matmul(
                        ps[0:D, 512 * s : 512 * (s + 1)],
                        wk2_sb[:, D * hk : D * (hk + 1)],
                        b_sb["k"][:, 512 * s : 512 * (s + 1)],
                    )
                pv = ps[0:D, 0:N].rearrange("p (b t c) -> p b t c", t=2, c=128)
                nc.vector.tensor_copy(
                    kT_sb[0:D, 512 * hk : 512 * (hk + 1)], pv[:, :, 0, :]
                )
                nc.vector.tensor_copy(
                    kT_sb[D:128, 512 * hk : 512 * (hk + 1)], pv[:, :, 1, :]
                )

            # ---- v natural (128 keys, d) per key chunk + ones column ----
            # chunk c (= 8*hk + pb) rows: keys [128c, 128c+128) of (hk, pos)
            v_sb = bigs.tile([128, KCH, D + 1], f32r)
            nc.vector.memset(v_sb.bitcast(f32)[:, :, D : D + 1], 0.0)
            nc.scalar.add(
                v_sb[:, :, D : D + 1], v_sb.bitcast(f32)[:, :, D : D + 1], 1.0
            )
            vv = v_sb.rearrange("p (h pb) e -> p pb h e", pb=8)
            for pb in range(8):
                ps = mpsum.tile([128, 1024], f32, tag="m")
                for hk in range(HEADS):
                    nc.tensor.matmul(
                        ps[:, D * hk : D * (hk + 1)],
                        b_sb["v"][:, 128 * pb : 128 * (pb + 1)],
                        wv2_sb[:, D * hk : D * (hk + 1)],
                    )
                nc.vector.tensor_copy(
                    vv[:, pb, :, 0:D],
                    ps[:, 0:INNER].rearrange("p (h e) -> p h e", h=HEADS),
                )

            # ---- attention: per query tile, streamed over key-chunk triples ----
            # (FD=1536 exp amortizes ACT's fixed per-instruction cost; 64
            #  chunks = 21 triples + 1 leftover). Each q-tile's finalize is
            #  deferred into the next q-tile's loop so the broadcast matmul
            #  never stalls the S/AV stream on PE.
            oT_sb = bigs.tile([128, 2, N], f32r)  # [64*(hl%2)+d, hl//2, pos]
            GROUPS = [(2 * g, 2) for g in range(KCH // 2)]

            def finalize(qt, av):
                # normalize: o^T = av[0:64] / av[64] (per-query column);
                # broadcast 1/l across partitions with a K=1 ones matmul.
                hl, s = divmod(qt, 2)
                rq = small.tile([1, QT], f32, tag="rq")
                nc.vector.reciprocal(rq[:], av[D : D + 1, :])
                bc_ps = mpsum.tile([128, 1024], f32, tag="m")
                nc.tensor.matmul(bc_ps[0:D, 0:QT], ones_sb[:], rq[:])
                rb = small.tile([D, QT], f32, tag="rb")
                nc.vector.tensor_copy(rb[:], bc_ps[0:D, 0:QT])
                pr, row = divmod(hl, 2)
                nc.vector.tensor_mul(
                    oT_sb[D * row : D * (row + 1), pr, QT * s : QT * (s + 1)],
                    av[0:D, :],
                    rb[:],
                )

            pending = None  # (qt, av) awaiting finalize
            for qt in range(NQT):
                hl, s = divmod(qt, 2)
                qlo = qT_sb[0:D, N * hl + QT * s : N * hl + QT * (s + 1)]
                qhi = qT_sb[D:128, N * hl + QT * s : N * hl + QT * (s + 1)]
                av = avpsum.tile([D + 1, QT], f32)
                prev = None  # (P tile, c0, cn) awaiting A@V
                for g in range(len(GROUPS) + 1):
                    if g < len(GROUPS):
                        c0, cn = GROUPS[g]
                        ps = mpsum.tile([128, 1024], f32, tag="m")
                        # chunk pair runs concurrently: row groups 0-1 / 2-3
                        nc.tensor.matmul(
                            ps[:, 0:512], kT_sb[0:D, 128 * g : 128 * (g + 1)], qlo
                        )
                        nc.tensor.matmul(
                            ps[:, 512:1024],
                            kT_sb[D:128, 128 * g : 128 * (g + 1)],
                            qhi,
                        )
                        pt = ppool.tile([128, 1024], f32r, tag="P")
                        nc.scalar.activation(
                            pt[:, 0 : 512 * cn], ps[:, 0 : 512 * cn], AF.Exp
                        )
                        nxt = (pt, c0, cn)
                    else:
                        nxt = None
                    if g == 1 and pending is not None:
                        finalize(*pending)
                        pending = None
                    if prev is not None:
                        pt, c0, cn = prev
                        for j in range(cn):
                            c = c0 + j
                            nc.tensor.matmul(
                                av[:],
                                v_sb[:, c, :],
                                pt[:, 512 * j : 512 * (j + 1)],
                                start=(c == 0),
                                stop=(c == KCH - 1),
                            )
                    prev = nxt
                pending = (qt, av)
            finalize(*pending)

            # ---- output projection: partial^T = wog^T @ o^T  (256, 1024) ----
            o_out = bigs.tile([128, 2, N], f32)
            for f in range(2):
                ps = mpsum.tile([128, 1024], f32, tag="m")
                for s2 in range(2):
                    for p in range(2):
                        nc.tensor.matmul(
                            ps[:, 512 * s2 : 512 * (s2 + 1)],
                            wog_sb[:, p, 128 * f : 128 * (f + 1)],
                            oT_sb[:, p, 512 * s2 : 512 * (s2 + 1)],
                            start=(p == 0),
                            stop=(p == 1),
                        )
                nc.vector.tensor_copy(o_out[:, f, :], ps[:, 0:N])
                nc.sync.dma_start(out[128 * f : 128 * (f + 1), :], o_out[:, f, :])

    nc.compile()
    return nc


def _get_nc():
    if "nc" not in _BUILT:
        _BUILT["nc"] = _build()
    return _BUILT["nc"]


def shard_inputs(x, wq1, wq2, wk1, wk2, wv1, wv2, wo, bo):
    """Full inputs -> list of 8 per-core input maps."""
    c = np.ascontiguousarray
    x = np.asarray(x, np.float32)
    in_maps = []
    for core in range(NCORES):
        b, g = divmod(core, 2)
        in_maps.append(
            {
                "xT": c(x[b].T.astype(np.float32)),
                "wq1": c(np.asarray(wq1, np.float32)),
                "wk1": c(np.asarray(wk1, np.float32)),
                "wv1": c(np.asarray(wv1, np.float32)),
                "wq2g": c(np.asarray(wq2, np.float32)[:, 256 * g : 256 * (g + 1)]),
                "wk2": c(np.asarray(wk2, np.float32)),
                "wv2": c(np.asarray(wv2, np.float32)),
                "wog": c(np.asarray(wo, np.float32)[256 * g : 256 * (g + 1), :]),
            }
        )
    return in_maps


def unshard_output(results, bo):
    """8 per-core partial^T (256, 1024) -> full (4, 1024, 256) output."""
    bo = np.asarray(bo, np.float32)
    out = np.empty((B, N, QS), np.float32)
    for b in range(B):
        acc = results[2 * b]["out"] + results[2 * b + 1]["out"]  # (256, 1024)
        out[b] = acc.T + bo
    return out


def kernel(x, wq1, wq2, wk1, wk2, wv1, wv2, wo, bo):
    from concourse.bass_utils import run_bass_kernel_spmd

    nc = _get_nc()
    in_maps = shard_inputs(x, wq1, wq2, wk1, wk2, wv1, wv2, wo, bo)
    res = run_bass_kernel_spmd(nc, in_maps, core_ids=list(range(NCORES)))
    return unshard_output(res.results, bo)



# revision 4
# speedup vs baseline: 1.0062x; 1.0062x over previous
"""Trainium2 Bass kernel for nn_CrossAttention_15418932593009.

Reference computation (fp32):
    q = (x @ wq1) @ wq2                      # (b, n, h*d), bottleneck 40
    k = silu(x @ wk1) @ wk2
    v = (x @ wv1) @ wv2
    split '(b n (h d)) -> (b (h n) d)'       # heads folded into sequence!
    sim  = q @ k.T * d**-0.5                 # (b, h*n, h*n) = (4, 8192, 8192)
    attn = softmax(sim, axis=-1)
    out  = attn @ v                          # (b, h*n, d)
    merge back -> (b, n, h*d); out @ wo + bo

Sharding: 8 cores = 4 batches x 2 query-head groups (heads 0-3 / 4-7).
Each core computes full K/V for its batch (all 8 heads) and attention for
its 4 query heads (4096 query rows x 8192 keys), then its partial of the
output projection. Host sums the two partials per batch and adds bo.

Per-core design (cost-model-driven):
- Scores S^T tiles [128 keys, 512 queries] from fp32r matmuls (the q
  projection is pre-scaled by d**-0.5 * log2(e) on the host, so scores
  arrive in base-2).
- exp is split across two engines: ~70% of score tiles on ACT
  (activation Exp, scale=ln2, bf16 out) and ~30% on DVE via a two-pass
  Schraudolph: pass1 = tensor_scalar int32 writeback y=int(s*2^23+bias)
  whose bits reinterpret as 2^k*(1+f); pass2 = one custom DVE op that
  extracts m=1+f with AND/OR bit ops and multiplies by a deg-2
  correction poly 2^(m-1)/m (max rel err ~3.5e-3, validated on HW).
- A@V runs transposed-free-dim: out[128 queries, 65] = P^T.T @ V with
  bf16 P^T slices as the stationary operand and V (with a ones column
  for the softmax denominator) as the 65-wide moving operand; 64 key
  chunks accumulate into one PSUM bank holding all 4 query-chunk
  accumulators (single start/stop per bank).
- Normalize on DVE (per-partition reciprocal scalar), transpose o via
  PE transpose-mode, final projection as natural-layout [pos, 256]
  matmuls; output DMA'd untransposed.
"""

import numpy as np

HEADS = 8
D = 64
BOT = 40
B = 4
N = 1024
QS = 256
INNER = HEADS * D          # 512
GH = 4                     # query heads per core
KCH = HEADS * N // 128     # 64 key chunks of 128
NQT = 8                    # 8 query tiles of 512 per core
NCORES = 8

LOG2E = float(np.log2(np.e))
LN2 = float(np.log(2.0))
MAGIC = float(127 * (1 << 23))     # Schraudolph bias (bits of 1.0f)
SCALE23 = float(1 << 23)
# deg-2 minimax of corr(m) = 2^(m-1)/m on [1,2): out = ((A2*m+A1)*m+A0)*P~
CA2, CA1, CA0 = 0.23375693, -0.69456113, 1.45744953

# exp tiles handled by DVE (two-pass) instead of ACT: i % 10 in this set
DVE_EXP_SLOTS = frozenset({1, 4, 7})

_BUILT = {}


def _register_exp_op():
    """Register the Schraudolph-correction custom DVE op (idempotent)."""
    import concourse.dve_ops as dve_ops
    from concourse.dve_spec import (
        AluOp, Bin, C0, C1, C2, C3, One, Spec, Src0,
        _has_src1, _spill_c3_to_src1, lower,
    )
    from concourse.dve_table_gen import dve_ver_for
    from concourse.dve_uop import DveOpSpec

    NAME = "EXP_SCHRAUD_CORR_ANT"
    if NAME in dve_ops._SUB_OPCODE_FOR_NAME:
        return next(op for op in dve_ops.OPS if op.name == NAME)

    # m = bitcast((bits(P~) & 0x007FFFFF) | bits(1.0)) = 1 + frac in [1,2)
    m = Bin(AluOp.BITWISE_OR, Bin(AluOp.BITWISE_AND, Src0, C0), One)
    body = _spill_c3_to_src1(((C1 * m + C2) * m + C3) * Src0)

    def ref(in0, in1, c0, c1, c2):
        bits = in0.view(np.int32)
        mm = ((bits & np.int32(0x007FFFFF)) | np.int32(0x3F800000)).view(
            np.float32
        )
        return ((c1 * mm + c2) * mm + in1) * in0

    spec = Spec(body=body, reference=ref)
    row = max(dve_ops._SUB_OPCODE_FOR_NAME.values()) + 1
    assert row < 0x20
    ver = dve_ver_for("TRN2")
    lowered = DveOpSpec(name=NAME, opcode=row, uops=lower(spec, ver=ver),
                        rd1_en=_has_src1(spec))
    op = dve_ops.DveOp(NAME, spec, subdim=False, uops_sha={ver: lowered.sha(ver)})
    dve_ops.OPS.append(op)
    dve_ops._SUB_OPCODE_FOR_NAME[NAME] = row
    dve_ops.CUSTOM_DVE_SPECS[NAME] = spec
    return op


def _build():
    """Build the single-core Bass module (same NEFF for all 8 cores)."""
    import concourse.bass as bass
    import concourse.mybir as mybir
    import concourse.tile as tile
    from concourse import bacc

    exp_op = _register_exp_op()

    dt = mybir.dt
    f32 = dt.float32
    f32r = dt.float32r
    bf16 = dt.bfloat16
    i32 = dt.int32
    AF = mybir.ActivationFunctionType
    Alu = mybir.AluOpType
    PSUM = bass.MemorySpace.PSUM

    nc = bacc.Bacc()

    # ---- DRAM I/O (per core); float32r = same bits as fp32 ----
    xT = nc.dram_tensor("xT", [QS, N], f32r, kind="ExternalInput")     # x[b].T
    wq1 = nc.dram_tensor("wq1", [QS, BOT], f32r, kind="ExternalInput")  # prescaled
    wk1 = nc.dram_tensor("wk1", [QS, BOT], f32r, kind="ExternalInput")
    wv1 = nc.dram_tensor("wv1", [QS, BOT], f32r, kind="ExternalInput")
    wq2g = nc.dram_tensor("wq2g", [BOT, GH * D], f32r, kind="ExternalInput")
    wk2 = nc.dram_tensor("wk2", [BOT, INNER], f32r, kind="ExternalInput")
    wv2 = nc.dram_tensor("wv2", [BOT, INNER], f32r, kind="ExternalInput")
    wog = nc.dram_tensor("wog", [GH * D, QS], f32r, kind="ExternalInput")
    ident = nc.dram_tensor("ident", [128, 128], f32r, kind="ExternalInput")
    out = nc.dram_tensor("out", [N, QS], f32, kind="ExternalOutput")  # natural

    with tile.TileContext(nc) as tc:
        with (
            tc.tile_pool(name="consts", bufs=1) as consts,
            tc.tile_pool(name="bigs", bufs=1) as bigs,
            tc.tile_pool(name="pp", bufs=6) as ppool,
            tc.tile_pool(name="yy", bufs=2) as ypool,
            tc.tile_pool(name="small", bufs=8) as small,
            tc.tile_pool(name="sp", bufs=2, space=PSUM) as sppool,
            tc.tile_pool(name="avp", bufs=2, space=PSUM) as avpool,
            tc.tile_pool(name="mp", bufs=2, space=PSUM) as mppool,
        ):
            # ---- load inputs ----
            xT_sb = bigs.tile([128, 2, N], f32r)
            for cc in range(2):
                nc.sync.dma_start(
                    xT_sb[:, cc, :],
                    xT.rearrange("(c p) n -> p c n", c=2)[:, cc, :],
                )

            w1_sb = {}
            for name, t in (("q", wq1), ("k", wk1), ("v", wv1)):
                w = consts.tile([128, 2, BOT], f32r, name=f"w{name}1_sb")
                nc.sync.dma_start(w[:], t.rearrange("(c p) n -> p c n", c=2))
                w1_sb[name] = w
            wq2_sb = consts.tile([BOT, GH * D], f32r)
            nc.sync.dma_start(wq2_sb[:], wq2g[:])
            wk2_sb = consts.tile([BOT, INNER], f32r)
            nc.sync.dma_start(wk2_sb[:], wk2[:])
            wv2_sb = consts.tile([BOT, INNER], f32r)
            nc.sync.dma_start(wv2_sb[:], wv2[:])
            wog_sb = consts.tile([D, GH, QS], f32r)
            nc.sync.dma_start(wog_sb[:], wog.rearrange("(h p) n -> p h n", h=GH))
            ident_sb = consts.tile([128, 128], f32r)
            nc.sync.dma_start(ident_sb[:], ident[:])

            mask_sb = consts.tile([128, 1], i32)
            nc.vector.memset(mask_sb[:], 0x007FFFFF)
            a0_sb = consts.tile([128, 1], f32)
            nc.vector.memset(a0_sb[:], CA0)

            # ---- bottleneck projections: bX^T = wX1^T @ x^T  (40, 1024) ----
            b_sb = {}
            for name in ("q", "k", "v"):
                ps = sppool.tile([128, 1024], f32, tag="sp")
                for s in range(2):
                    for cc in range(2):
                        nc.tensor.matmul(
                            ps[0:BOT, 512 * s : 512 * (s + 1)],
                            w1_sb[name][:, cc, :],
                            xT_sb[:, cc, 512 * s : 512 * (s + 1)],
                            start=(cc == 0),
                            stop=(cc == 1),
                        )
                bt = bigs.tile([BOT, N], f32r, name=f"b{name}_sb")
                if name == "k":
                    # silu(x) = x * sigmoid(x)
                    sg = bigs.tile([BOT, N], f32, name="sg_sb")
                    nc.scalar.activation(sg[:], ps[0:BOT, 0:N], AF.Sigmoid)
                    nc.vector.tensor_mul(bt[:], ps[0:BOT, 0:N], sg[:])
                else:
                    nc.vector.tensor_copy(bt[:], ps[0:BOT, 0:N])
                b_sb[name] = bt

            # ---- q^T (64, 4096): head-major columns (scaled upstream) ----
            qT_sb = bigs.tile([D, GH * N], f32r)
            for hl in range(GH):
                ps = sppool.tile([128, 1024], f32, tag="sp")
                for s in range(2):
                    nc.tensor.matmul(
                        ps[0:D, 512 * s : 512 * (s + 1)],
                        wq2_sb[:, D * hl : D * (hl + 1)],
                        b_sb["q"][:, 512 * s : 512 * (s + 1)],
                    )
                nc.scalar.activation(
                    qT_sb[:, N * hl : N * (hl + 1)], ps[0:D, 0:N], AF.Copy
                )

            # ---- k^T (64, 8192): key-index columns ----
            kT_sb = bigs.tile([D, HEADS * N], f32r)
            for hk in range(HEADS):
                ps = sppool.tile([128, 1024], f32, tag="sp")
                for s in range(2):
                    nc.tensor.matmul(
                        ps[0:D, 512 * s : 512 * (s + 1)],
                        wk2_sb[:, D * hk : D * (hk + 1)],
                        b_sb["k"][:, 512 * s : 512 * (s + 1)],
                    )
                nc.scalar.activation(
                    kT_sb[:, N * hk : N * (hk + 1)], ps[0:D, 0:N], AF.Copy
                )

            # ---- v natural (128 keys, kch, d+1) bf16 with ones column ----
            v_sb = bigs.tile([128, KCH, D + 1], bf16)
            nc.vector.memset(v_sb[:, :, D : D + 1], 1.0)
            vv = v_sb.rearrange("p (h pb) e -> p pb h e", pb=8)
            for pb in range(8):
                ps = sppool.tile([128, 1024], f32, tag="sp")
                nc.tensor.matmul(
                    ps[:, 0:INNER],
                    b_sb["v"][:, 128 * pb : 128 * (pb + 1)],
                    wv2_sb[:],
                )
                nc.vector.tensor_copy(
                    vv[:, pb, :, 0:D],
                    ps[:, 0:INNER].rearrange("p (h e) -> p h e", h=HEADS),
                )

            # ---- attention stream ----
            o_sb = bigs.tile([128, 4 * NQT, D], f32r)   # normalized o, natural
            oT_sb = bigs.tile([D, GH * N], f32r)        # o^T, (head, pos) cols
            out_sb = bigs.tile([128, 8, QS], f32)

            def normalize(qt, av):
                # o = av[:, 0:64] / av[:, 64] per query-chunk, bf16-ish f32r
                hl, s = divmod(qt, 2)
                rq = small.tile([128, 4, 1], f32, tag="rq")
                av4 = av.rearrange("p (q e) -> p q e", q=4)
                nc.vector.reciprocal(rq[:], av4[:, :, D : D + 1])
                for qc in range(4):
                    qcg = hl * 8 + s * 4 + qc
                    nc.vector.tensor_scalar(
                        o_sb[:, qcg, :],
                        av[:, 65 * qc : 65 * qc + D],
                        rq[:, qc, :],
                        None,
                        op0=Alu.mult,
                    )

            def transposes(qt):
                hl, s = divmod(qt, 2)
                for qc in range(4):
                    qcg = hl * 8 + s * 4 + qc
                    tp = mppool.tile([128, 512], f32, tag="mp")
                    nc.tensor.matmul(
                        tp.bitcast(f32r)[0:D, 0:128],
                        o_sb[:, qcg, :],
                        ident_sb[:],
                        is_transpose=True,
                    )
                    nc.vector.tensor_copy(
                        oT_sb[:, N * hl + 512 * s + 128 * qc :
                              N * hl + 512 * s + 128 * qc + 128],
                        tp.bitcast(f32r)[0:D, 0:128],
                    )

            def final_block(pb):
                fp = mppool.tile([128, 512], f32, tag="mp")
                for hl in range(GH):
                    nc.tensor.matmul(
                        fp[:, 0:QS],
                        oT_sb[:, N * hl + 128 * pb : N * hl + 128 * pb + 128],
                        wog_sb[:, hl, :],
                        start=(hl == 0),
                        stop=(hl == GH - 1),
                    )
                nc.vector.tensor_copy(out_sb[:, pb, :], fp[:, 0:QS])
                nc.sync.dma_start(
                    out[128 * pb : 128 * (pb + 1), :], out_sb[:, pb, :]
                )

            pending = None  # (qt, av) awaiting normalize/transpose
            for qt in range(NQT):
                hl, s = divmod(qt, 2)
                qcol = N * hl + 512 * s
                av = avpool.tile([128, 4 * (D + 1)], f32)
                for p in range(32):
                    i = qt * 32 + p
                    sp = sppool.tile([128, 1024], f32, tag="sp")
                    for j in range(2):
                        c = 2 * p + j
                        nc.tensor.matmul(
                            sp[:, 512 * j : 512 * (j + 1)],
                            kT_sb[:, 128 * c : 128 * (c + 1)],
                            qT_sb[:, qcol : qcol + 512],
                        )
                    pt = ppool.tile([128, 1024], bf16, tag="P")
                    if (i % 10) in DVE_EXP_SLOTS:
                        y = ypool.tile([128, 1024], i32, tag="y")
                        nc.vector.tensor_scalar(
                            y[:], sp[:], SCALE23, MAGIC,
                            op0=Alu.mult, op1=Alu.add,
                        )
                        nc.vector._custom_dve(
                            exp_op, out=pt[:], in0=y.bitcast(f32)[:],
                            in1=a0_sb[:], s0=mask_sb.bitcast(f32)[:],
                            s1=CA2, imm2=CA1,
                        )
                    else:
                        nc.scalar.activation(pt[:], sp[:], AF.Exp, scale=LN2)
                    if p == 2 and pending is not None:
                        normalize(*pending)
                    if p == 8 and pending is not None:
                        transposes(pending[0])
                        pending = None
                        if qt == NQT - 1:
                            for pb in range(4):
                                final_block(pb)
                    for j in range(2):
                        c = 2 * p + j
                        for qc in range(4):
                            nc.tensor.matmul(
                                av[:, 65 * qc : 65 * qc + D + 1],
                                pt[:, 512 * j + 128 * qc :
                                   512 * j + 128 * qc + 128],
                                v_sb[:, c, :],
                                start=(c == 0 and qc == 0),
                                stop=(c == KCH - 1 and qc == 3),
                                skip_group_check=True,
                            )
                pending = (qt, av)
            normalize(*pending)
            transposes(pending[0])
            for pb in range(4, 8):
                final_block(pb)

    nc.compile()
    return nc


def _get_nc():
    if "nc" not in _BUILT:
        _BUILT["nc"] = _build()
    return _BUILT["nc"]


def shard_inputs(x, wq1, wq2, wk1, wk2, wv1, wv2, wo, bo):
    """Full inputs -> list of 8 per-core input maps."""
    c = np.ascontiguousarray
    x = np.asarray(x, np.float32)
    # fold attention scale and base-2 conversion into the q path
    wq1s = np.asarray(wq1, np.float32) * np.float32(D**-0.5 * LOG2E)
    eye = np.eye(128, dtype=np.float32)
    in_maps = []
    for core in range(NCORES):
        b, g = divmod(core, 2)
        in_maps.append(
            {
                "xT": c(x[b].T.astype(np.float32)),
                "wq1": c(wq1s),
                "wk1": c(np.asarray(wk1, np.float32)),
                "wv1": c(np.asarray(wv1, np.float32)),
                "wq2g": c(np.asarray(wq2, np.float32)[:, 256 * g : 256 * (g + 1)]),
                "wk2": c(np.asarray(wk2, np.float32)),
                "wv2": c(np.asarray(wv2, np.float32)),
                "wog": c(np.asarray(wo, np.float32)[256 * g : 256 * (g + 1), :]),
                "ident": eye,
            }
        )
    return in_maps


def unshard_output(results, bo):
    """8 per-core partial (1024, 256) -> full (4, 1024, 256) output."""
    bo = np.asarray(bo, np.float32)
    out = np.empty((B, N, QS), np.float32)
    for b in range(B):
        out[b] = results[2 * b]["out"] + results[2 * b + 1]["out"] + bo
    return out


def kernel(x, wq1, wq2, wk1, wk2, wv1, wv2, wo, bo):
    from concourse.bass_utils import run_bass_kernel_spmd

    nc = _get_nc()
    in_maps = shard_inputs(x, wq1, wq2, wk1, wk2, wv1, wv2, wo, bo)
    res = run_bass_kernel_spmd(nc, in_maps, core_ids=list(range(NCORES)))
    return unshard_output(res.results, bo)


# revision 11
# speedup vs baseline: 1.2184x; 1.2110x over previous
"""Trainium2 Bass kernel for nn_CrossAttention_15418932593009.

Reference computation (fp32):
    q = (x @ wq1) @ wq2                      # (b, n, h*d), bottleneck 40
    k = silu(x @ wk1) @ wk2
    v = (x @ wv1) @ wv2
    split '(b n (h d)) -> (b (h n) d)'       # heads folded into sequence!
    sim  = q @ k.T * d**-0.5                 # (b, h*n, h*n) = (4, 8192, 8192)
    attn = softmax(sim, axis=-1)
    out  = attn @ v                          # (b, h*n, d)
    merge back -> (b, n, h*d); out @ wo + bo

Sharding: 8 cores = 4 batches x 2 query-head groups (heads 0-3 / 4-7).
Each core computes full K/V for its batch (all 8 heads) and attention for
its 4 query heads (4096 query rows x 8192 keys), then its partial of the
output projection. Host sums the two partials per batch and adds bo.

Per-core design (cost-model-driven):
- Scores S^T tiles [128 keys, 512 queries] from fp32r matmuls (the q
  projection is pre-scaled by d**-0.5 * log2(e) on the host, so scores
  arrive in base-2).
- exp is split across two engines: ~70% of score tiles on ACT
  (activation Exp, scale=ln2, bf16 out) and ~30% on DVE via a two-pass
  Schraudolph: pass1 = tensor_scalar int32 writeback y=int(s*2^23+bias)
  whose bits reinterpret as 2^k*(1+f); pass2 = one custom DVE op that
  extracts m=1+f with AND/OR bit ops and multiplies by a deg-2
  correction poly 2^(m-1)/m (max rel err ~3.5e-3, validated on HW).
- A@V runs transposed-free-dim: out[128 queries, 65] = P^T.T @ V with
  bf16 P^T slices as the stationary operand and V (with a ones column
  for the softmax denominator) as the 65-wide moving operand; 64 key
  chunks accumulate into one PSUM bank holding all 4 query-chunk
  accumulators (single start/stop per bank).
- Normalize on DVE (per-partition reciprocal scalar), transpose o via
  PE transpose-mode, final projection as natural-layout [pos, 256]
  matmuls; output DMA'd untransposed.
"""

import numpy as np

HEADS = 8
D = 64
BOT = 40
B = 4
N = 1024
QS = 256
INNER = HEADS * D          # 512
GH = 4                     # query heads per core
KCH = HEADS * N // 128     # 64 key chunks of 128
NQT = 8                    # 8 query tiles of 512 per core
NCORES = 8

LOG2E = float(np.log2(np.e))
LN2 = float(np.log(2.0))
MAGIC = float(127 * (1 << 23))     # Schraudolph bias (bits of 1.0f)
SCALE23 = float(1 << 23)
# deg-2 minimax of corr(m) = 2^(m-1)/m on [1,2): out = ((A2*m+A1)*m+A0)*P~
CA2, CA1, CA0 = 0.23375693, -0.69456113, 1.45744953

# exp tiles handled by DVE (two-pass) instead of ACT: i % 10 in this set
DVE_EXP_SLOTS = frozenset({1, 4, 7})

_BUILT = {}


def _register_exp_op():
    """Register the Schraudolph-correction custom DVE op (idempotent)."""
    import concourse.dve_ops as dve_ops
    from concourse.dve_spec import (
        AluOp, Bin, C0, C1, C2, C3, One, Spec, Src0,
        _has_src1, _spill_c3_to_src1, lower,
    )
    from concourse.dve_table_gen import dve_ver_for
    from concourse.dve_uop import DveOpSpec

    NAME = "EXP_SCHRAUD_CORR_ANT"
    if NAME in dve_ops._SUB_OPCODE_FOR_NAME:
        return next(op for op in dve_ops.OPS if op.name == NAME)

    # m = bitcast((bits(P~) & 0x007FFFFF) | bits(1.0)) = 1 + frac in [1,2)
    m = Bin(AluOp.BITWISE_OR, Bin(AluOp.BITWISE_AND, Src0, C0), One)
    body = _spill_c3_to_src1(((C1 * m + C2) * m + C3) * Src0)

    def ref(in0, in1, c0, c1, c2):
        bits = in0.view(np.int32)
        mm = ((bits & np.int32(0x007FFFFF)) | np.int32(0x3F800000)).view(
            np.float32
        )
        return ((c1 * mm + c2) * mm + in1) * in0

    spec = Spec(body=body, reference=ref)
    row = max(dve_ops._SUB_OPCODE_FOR_NAME.values()) + 1
    assert row < 0x20
    ver = dve_ver_for("TRN2")
    lowered = DveOpSpec(name=NAME, opcode=row, uops=lower(spec, ver=ver),
                        rd1_en=_has_src1(spec))
    op = dve_ops.DveOp(NAME, spec, subdim=False, uops_sha={ver: lowered.sha(ver)})
    dve_ops.OPS.append(op)
    dve_ops._SUB_OPCODE_FOR_NAME[NAME] = row
    dve_ops.CUSTOM_DVE_SPECS[NAME] = spec
    return op


def _build():
    """Build the single-core Bass module (same NEFF for all 8 cores)."""
    import concourse.bass as bass
    import concourse.mybir as mybir
    import concourse.tile as tile
    from concourse import bacc

    exp_op = _register_exp_op()

    dt = mybir.dt
    f32 = dt.float32
    f32r = dt.float32r
    bf16 = dt.bfloat16
    i32 = dt.int32
    AF = mybir.ActivationFunctionType
    Alu = mybir.AluOpType
    PSUM = bass.MemorySpace.PSUM

    nc = bacc.Bacc()

    # ---- DRAM I/O (per core); float32r = same bits as fp32 ----
    xT = nc.dram_tensor("xT", [QS, N], f32r, kind="ExternalInput")     # x[b].T
    wq1 = nc.dram_tensor("wq1", [QS, BOT], f32r, kind="ExternalInput")  # prescaled
    wk1 = nc.dram_tensor("wk1", [QS, BOT], f32r, kind="ExternalInput")
    wv1 = nc.dram_tensor("wv1", [QS, BOT], f32r, kind="ExternalInput")
    wq2g = nc.dram_tensor("wq2g", [BOT, GH * D], f32r, kind="ExternalInput")
    wk2 = nc.dram_tensor("wk2", [BOT, INNER], f32r, kind="ExternalInput")
    wv2 = nc.dram_tensor("wv2", [BOT, INNER], f32r, kind="ExternalInput")
    wog = nc.dram_tensor("wog", [GH * D, QS], f32r, kind="ExternalInput")
    ident = nc.dram_tensor("ident", [128, 128], f32r, kind="ExternalInput")
    out = nc.dram_tensor("out", [N, QS], f32, kind="ExternalOutput")  # natural

    with tile.TileContext(nc) as tc:
        with (
            tc.tile_pool(name="consts", bufs=1) as consts,
            tc.tile_pool(name="bigs", bufs=1) as bigs,
            tc.tile_pool(name="pp", bufs=6) as ppool,
            tc.tile_pool(name="yy", bufs=2) as ypool,
            tc.tile_pool(name="small", bufs=8) as small,
            tc.tile_pool(name="sp", bufs=3, space=PSUM) as sppool,
            tc.tile_pool(name="avp", bufs=2, space=PSUM) as avpool,
        ):
            # ---- load inputs ----
            xT_sb = bigs.tile([128, 2, N], f32r)
            for cc in range(2):
                nc.sync.dma_start(
                    xT_sb[:, cc, :],
                    xT.rearrange("(c p) n -> p c n", c=2)[:, cc, :],
                )

            w1_sb = {}
            for name, t in (("q", wq1), ("k", wk1), ("v", wv1)):
                w = consts.tile([128, 2, BOT], f32r, name=f"w{name}1_sb")
                nc.sync.dma_start(w[:], t.rearrange("(c p) n -> p c n", c=2))
                w1_sb[name] = w
            wq2_sb = consts.tile([BOT, GH * D], f32r)
            nc.sync.dma_start(wq2_sb[:], wq2g[:])
            wk2_sb = consts.tile([BOT, INNER], f32r)
            nc.sync.dma_start(wk2_sb[:], wk2[:])
            wv2_sb = consts.tile([BOT, INNER], f32r)
            nc.sync.dma_start(wv2_sb[:], wv2[:])
            wog_sb = consts.tile([D, GH, QS], f32r)
            nc.sync.dma_start(wog_sb[:], wog.rearrange("(h p) n -> p h n", h=GH))
            ident_sb = consts.tile([128, 128], f32r)
            nc.sync.dma_start(ident_sb[:], ident[:])

            mask_sb = consts.tile([128, 1], i32)
            nc.vector.memset(mask_sb[:], 0x007FFFFF)
            a0_sb = consts.tile([128, 1], f32)
            nc.vector.memset(a0_sb[:], CA0)

            # ---- bottleneck projections: bX^T = wX1^T @ x^T  (40, 1024) ----
            b_sb = {}
            for name in ("q", "k", "v"):
                ps = sppool.tile([128, 1024], f32, tag="sp")
                for s in range(2):
                    for cc in range(2):
                        nc.tensor.matmul(
                            ps[0:BOT, 512 * s : 512 * (s + 1)],
                            w1_sb[name][:, cc, :],
                            xT_sb[:, cc, 512 * s : 512 * (s + 1)],
                            start=(cc == 0),
                            stop=(cc == 1),
                        )
                bt = bigs.tile([BOT, N], f32r, name=f"b{name}_sb")
                if name == "k":
                    # silu(x) = x * sigmoid(x)
                    sg = bigs.tile([BOT, N], f32, name="sg_sb")
                    nc.scalar.activation(sg[:], ps[0:BOT, 0:N], AF.Sigmoid)
                    nc.vector.tensor_mul(bt[:], ps[0:BOT, 0:N], sg[:])
                else:
                    nc.vector.tensor_copy(bt[:], ps[0:BOT, 0:N])
                b_sb[name] = bt

            # ---- q^T (64, 4096): head-major columns (scaled upstream) ----
            qT_sb = bigs.tile([D, GH * N], f32r)
            for hl in range(GH):
                ps = sppool.tile([128, 1024], f32, tag="sp")
                for s in range(2):
                    nc.tensor.matmul(
                        ps[0:D, 512 * s : 512 * (s + 1)],
                        wq2_sb[:, D * hl : D * (hl + 1)],
                        b_sb["q"][:, 512 * s : 512 * (s + 1)],
                    )
                nc.scalar.activation(
                    qT_sb[:, N * hl : N * (hl + 1)], ps[0:D, 0:N], AF.Copy
                )

            # ---- k^T (64, 8192): key-index columns ----
            kT_sb = bigs.tile([D, HEADS * N], f32r)
            for hk in range(HEADS):
                ps = sppool.tile([128, 1024], f32, tag="sp")
                for s in range(2):
                    nc.tensor.matmul(
                        ps[0:D, 512 * s : 512 * (s + 1)],
                        wk2_sb[:, D * hk : D * (hk + 1)],
                        b_sb["k"][:, 512 * s : 512 * (s + 1)],
                    )
                nc.scalar.activation(
                    kT_sb[:, N * hk : N * (hk + 1)], ps[0:D, 0:N], AF.Copy
                )

            # ---- v natural (128 keys, kch, d+1) bf16 with ones column ----
            v_sb = bigs.tile([128, KCH, D + 1], bf16)
            nc.vector.memset(v_sb[:, :, D : D + 1], 1.0)
            vv = v_sb.rearrange("p (h pb) e -> p pb h e", pb=8)
            for pb in range(8):
                ps = sppool.tile([128, 1024], f32, tag="sp")
                nc.tensor.matmul(
                    ps[:, 0:INNER],
                    b_sb["v"][:, 128 * pb : 128 * (pb + 1)],
                    wv2_sb[:],
                )
                nc.vector.tensor_copy(
                    vv[:, pb, :, 0:D],
                    ps[:, 0:INNER].rearrange("p (h e) -> p h e", h=HEADS),
                )

            # ---- attention stream ----
            o_sb = bigs.tile([128, 4 * NQT, D], f32r)   # normalized o, natural
            oT_sb = bigs.tile([D, GH * N], f32r)        # o^T, (head, pos) cols
            out_sb = bigs.tile([128, 8, QS], f32)

            def normalize(qt, av):
                # o = av[:, 0:64] / av[:, 64] per query-chunk, bf16-ish f32r
                hl, s = divmod(qt, 2)
                rq = small.tile([128, 4, 1], f32, tag="rq")
                av4 = av.rearrange("p (q e) -> p q e", q=4)
                nc.vector.reciprocal(rq[:], av4[:, :, D : D + 1])
                for qc in range(4):
                    qcg = hl * 8 + s * 4 + qc
                    nc.vector.tensor_scalar(
                        o_sb[:, qcg, :],
                        av[:, 65 * qc : 65 * qc + D],
                        rq[:, qc, :],
                        None,
                        op0=Alu.mult,
                    )

            def transposes(qt):
                hl, s = divmod(qt, 2)
                tp = sppool.tile([128, 1024], f32, tag="sp")
                for qc in range(4):
                    qcg = hl * 8 + s * 4 + qc
                    nc.tensor.matmul(
                        tp.bitcast(f32r)[0:D, 128 * qc : 128 * qc + 128],
                        o_sb[:, qcg, :],
                        ident_sb[:],
                        is_transpose=True,
                    )
                    nc.vector.tensor_copy(
                        oT_sb[:, N * hl + 512 * s + 128 * qc :
                              N * hl + 512 * s + 128 * qc + 128],
                        tp.bitcast(f32r)[0:D, 128 * qc : 128 * qc + 128],
                    )

            def final_block(pb):
                fp = sppool.tile([128, 1024], f32, tag="sp")
                for hl in range(GH):
                    nc.tensor.matmul(
                        fp[:, 0:QS],
                        oT_sb[:, N * hl + 128 * pb : N * hl + 128 * pb + 128],
                        wog_sb[:, hl, :],
                        start=(hl == 0),
                        stop=(hl == GH - 1),
                    )
                nc.vector.tensor_copy(out_sb[:, pb, :], fp[:, 0:QS])
                nc.sync.dma_start(
                    out[128 * pb : 128 * (pb + 1), :], out_sb[:, pb, :]
                )

            from collections import deque

            AV_LAG = 2  # tiles of pipeline lag before A@V consumes P

            def emit_av(work):
                av, pt, p = work
                for j in range(2):
                    c = 2 * p + j
                    for qc in range(4):
                        nc.tensor.matmul(
                            av[:, 65 * qc : 65 * qc + D + 1],
                            pt[:, 512 * j + 128 * qc :
                               512 * j + 128 * qc + 128],
                            v_sb[:, c, :],
                            start=(c == 0 and qc == 0),
                            stop=(c == KCH - 1 and qc == 3),
                            skip_group_check=True,
                        )

            av_work = deque()
            pending = None  # (qt, av) awaiting normalize/transpose
            for qt in range(NQT):
                hl, s = divmod(qt, 2)
                qcol = N * hl + 512 * s
                avt = avpool.tile([128, 512], f32, tag="av")
                av = avt[:, 0 : 4 * (D + 1)]
                for p in range(32):
                    i = qt * 32 + p
                    sp = sppool.tile([128, 1024], f32, tag="sp")
                    for j in range(2):
                        c = 2 * p + j
                        nc.tensor.matmul(
                            sp[:, 512 * j : 512 * (j + 1)],
                            kT_sb[:, 128 * c : 128 * (c + 1)],
                            qT_sb[:, qcol : qcol + 512],
                        )
                    pt = ppool.tile([128, 1024], bf16, tag="P")
                    if (i % 10) in DVE_EXP_SLOTS:
                        y = ypool.tile([128, 1024], i32, tag="y")
                        nc.vector.tensor_scalar(
                            y[:], sp[:], SCALE23, MAGIC,
                            op0=Alu.mult, op1=Alu.add,
                        )
                        nc.vector._custom_dve(
                            exp_op, out=pt[:], in0=y.bitcast(f32)[:],
                            in1=a0_sb[:], s0=mask_sb.bitcast(f32)[:],
                            s1=CA2, imm2=CA1,
                        )
                    else:
                        nc.scalar.activation(pt[:], sp[:], AF.Exp, scale=LN2)
                    if p == 3 and pending is not None:
                        normalize(*pending)
                    if p == 8 and pending is not None:
                        transposes(pending[0])
                        pending = None
                    av_work.append((av, pt, p))
                    if len(av_work) > AV_LAG:
                        emit_av(av_work.popleft())
                pending = (qt, av)
            while av_work:
                emit_av(av_work.popleft())
            normalize(*pending)
            transposes(pending[0])
            for pb in range(8):
                final_block(pb)

    nc.compile()
    return nc


def _get_nc():
    if "nc" not in _BUILT:
        _BUILT["nc"] = _build()
    return _BUILT["nc"]


def shard_inputs(x, wq1, wq2, wk1, wk2, wv1, wv2, wo, bo):
    """Full inputs -> list of 8 per-core input maps."""
    c = np.ascontiguousarray
    x = np.asarray(x, np.float32)
    # fold attention scale and base-2 conversion into the q path
    wq1s = np.asarray(wq1, np.float32) * np.float32(D**-0.5 * LOG2E)
    eye = np.eye(128, dtype=np.float32)
    in_maps = []
    for core in range(NCORES):
        b, g = divmod(core, 2)
        in_maps.append(
            {
                "xT": c(x[b].T.astype(np.float32)),
                "wq1": c(wq1s),
                "wk1": c(np.asarray(wk1, np.float32)),
                "wv1": c(np.asarray(wv1, np.float32)),
                "wq2g": c(np.asarray(wq2, np.float32)[:, 256 * g : 256 * (g + 1)]),
                "wk2": c(np.asarray(wk2, np.float32)),
                "wv2": c(np.asarray(wv2, np.float32)),
                "wog": c(np.asarray(wo, np.float32)[256 * g : 256 * (g + 1), :]),
                "ident": eye,
            }
        )
    return in_maps


def unshard_output(results, bo):
    """8 per-core partial (1024, 256) -> full (4, 1024, 256) output."""
    bo = np.asarray(bo, np.float32)
    out = np.empty((B, N, QS), np.float32)
    for b in range(B):
        out[b] = results[2 * b]["out"] + results[2 * b + 1]["out"] + bo
    return out


def kernel(x, wq1, wq2, wk1, wk2, wv1, wv2, wo, bo):
    from concourse.bass_utils import run_bass_kernel_spmd

    nc = _get_nc()
    in_maps = shard_inputs(x, wq1, wq2, wk1, wk2, wv1, wv2, wo, bo)
    res = run_bass_kernel_spmd(nc, in_maps, core_ids=list(range(NCORES)))
    return unshard_output(res.results, bo)
